# revision 20
# baseline (speedup 1.0000x reference)
"""Trainium2 Bass kernel for nn_CameraViewTransformerLSS (LSS camera->BEV transformer).

Pipeline (B=1, N=6 cams, D=48 depth bins, 64x176 feature map, C=80 ctx channels,
128x128 BEV grid, 128 output channels):

  1. lift:    feat[n,d,h,w,c] = depth_prob[n,d,h,w] * context[n,c,h,w]
  2. splat:   scatter-add feat into BEV bins by frustum geometry
  3. head:    1x1 conv (80->128) + BN + ReLU

Key structural fact: with this camera rig (rotations about z only), the BEV bin
of a frustum point depends only on (camera n, depth d, image column w) -- NOT on
the image row h.  So the h axis can be contracted *before* any scatter:

  partial[(n,w,d), c] = sum_h depth[n,d,h,w] * ctx[n,c,h,w]     (a small matmul
  per camera-column "ray", K=h=64), reducing the scatter from 3.24M points to
  50688 points.

Two SPMD launches on 8 NeuronCores:
  L1 (ray-sharded):  each core lifts 132 of the 1056 rays via K=64 matmuls,
      4 rays packed per PE pass using the 4 array quadrants.
  host (free):       sort the 50688 partial rows by BEV bin into padded
      128-point K-tiles; snake-balance BEV rows across cores (16 rows each,
      uniform tile schedule so all cores run the identical program).
  L2 (bin-sharded):  each core scatter-accumulates its K-tiles into PSUM with
      one-hot matmuls (one-hot built on-device: iota == idx), then runs the
      1x1 conv + fused BN+ReLU and writes its 16 BEV rows.

The bin indices are computed on host with jnp mirroring the reference op
sequence exactly (a few points land exactly on bin boundaries; same backend =>
identical floor results).
"""

import functools

import numpy as np

import concourse.bacc as bacc
import concourse.mybir as mybir
import concourse.tile as tile
from concourse.bass_utils import run_bass_kernel_spmd

# ---------------------------------------------------------------- constants
NCAM, DD, HF, WF, CC = 6, 48, 64, 176, 80
BH = BW = 128
OC = 128
STRIDE = 4.0
PC = (-50.0, -50.0, -5.0, 50.0, 50.0, 3.0)
Z_MIN, Z_MAX = 1.0, 60.0
BN_EPS = 1e-5

NCORES = 8
RAYS = NCAM * WF            # 1056
RPC = RAYS // NCORES        # 132 rays per core
GPC = RPC // 4              # 33 groups of 4 rays
NSLOT = BH // NCORES        # 16 BEV rows per core
F32 = mybir.dt.float32
BF16 = mybir.dt.bfloat16

# dtype switches (validated for accuracy in test harness)
L1_BF16 = True              # lift matmul operands in bf16
L2_BF16 = True              # scatter matmul operands (partials + one-hot) in bf16
CONV_F32R = True            # 1x1 conv matmul in float32r (TF32-like, 4x faster)

_DT1 = BF16 if L1_BF16 else F32
_DT2 = BF16 if L2_BF16 else F32


def _np_dt(dt):
    if dt == BF16:
        import ml_dtypes

        return np.dtype(ml_dtypes.bfloat16)
    return np.dtype(np.float32)


# ---------------------------------------------------------------- L1 builder
DP = 64  # depth dim padded to 64 so matmul M=64 fills all PSUM partitions


@functools.lru_cache(maxsize=4)
def _build_l1(dt1):
    nc = bacc.Bacc("TRN2", target_bir_lowering=False, debug=False, num_devices=NCORES)
    d_in = nc.dram_tensor("d_in", [128, GPC * 2 * DP], dt1, kind="ExternalInput")
    c_in = nc.dram_tensor("c_in", [128, GPC * 2 * CC], dt1, kind="ExternalInput")
    part = nc.dram_tensor("part", [96, GPC * 2 * CC], F32, kind="ExternalOutput")

    NCH = 3                      # input DMA chunks
    GCH = GPC // NCH             # 11 groups per chunk
    BK = 512                     # fp32 elements per PSUM bank

    with tile.TileContext(nc) as tc:
        with (
            tc.tile_pool(name="din", bufs=2) as din_pool,
            tc.tile_pool(name="cin", bufs=2) as cin_pool,
            tc.tile_pool(name="stage", bufs=1) as stage_pool,
            tc.tile_pool(name="ps", bufs=4, space="PSUM") as ps_pool,
        ):
            stage = stage_pool.tile([128, GPC * 160], F32)
            for ch in range(NCH):
                dt_t = din_pool.tile([128, GCH * 2 * DP], dt1)
                nc.sync.dma_start(
                    out=dt_t[:], in_=d_in[:, ch * GCH * 2 * DP:(ch + 1) * GCH * 2 * DP]
                )
                ct_t = cin_pool.tile([128, GCH * 2 * CC], dt1)
                nc.sync.dma_start(
                    out=ct_t[:], in_=c_in[:, ch * GCH * 2 * CC:(ch + 1) * GCH * 2 * CC]
                )
                for gg in range(GCH):
                    g = ch * GCH + gg
                    # 2-bank PSUM tile: bank0 <- PE-row-0 rays, bank1 <- PE-row-64
                    # rays; column position picks the partition slice.
                    pt = ps_pool.tile([128, 2 * BK], F32, space="PSUM")
                    for pj in range(2):            # pair index within group
                        dsl = slice((2 * gg + pj) * DP, (2 * gg + pj + 1) * DP)
                        csl = slice((2 * gg + pj) * CC, (2 * gg + pj + 1) * CC)
                        psl = slice(pj * 64, pj * 64 + 64)       # partition slice
                        # ray 4g+pj (PE rows 0:64) -> bank 0
                        nc.tensor.matmul(
                            out=pt[psl, 0:CC],
                            lhsT=dt_t[0:64, dsl],
                            rhs=ct_t[0:64, csl],
                            start=True,
                            stop=True,
                        )
                        # ray 4g+2+pj (PE rows 64:128) -> bank 1
                        nc.tensor.matmul(
                            out=pt[psl, BK:BK + CC],
                            lhsT=dt_t[64:128, dsl],
                            rhs=ct_t[64:128, csl],
                            start=True,
                            stop=True,
                        )
                    src = pt[:, 0:2 * BK].rearrange("p (b x) -> p b x", b=2)[:, :, 0:CC]
                    dst = stage[:, g * 160:(g + 1) * 160].rearrange(
                        "p (b x) -> p b x", b=2
                    )
                    nc.vector.tensor_copy(out=dst, in_=src)
            nc.sync.dma_start(out=part[0:DD, :], in_=stage[0:DD, :])
            nc.sync.dma_start(out=part[DD:2 * DD, :], in_=stage[64:64 + DD, :])
    nc.compile()
    return nc


# Unpack map for L1 "part" output:
#   part rows 0:48  ("top", PSUM parts 0:48):   [:, g, 0] = ray 4g+0, [:, g, 1] = ray 4g+1
#   part rows 48:96 ("bot", PSUM parts 64:112): [:, g, 0] = ray 4g+2, [:, g, 1] = ray 4g+3
def _unpack_l1(out_core):
    S = out_core.reshape(96, GPC, 2, CC)
    top = S[0:DD]            # (48, 33, 2, 80)
    bot = S[DD:2 * DD]       # (48, 33, 2, 80)
    p = np.empty((RPC, DD, CC), np.float32)
    p[0::4] = top[:, :, 0].transpose(1, 0, 2)
    p[1::4] = top[:, :, 1].transpose(1, 0, 2)
    p[2::4] = bot[:, :, 0].transpose(1, 0, 2)
    p[3::4] = bot[:, :, 1].transpose(1, 0, 2)
    return p


# ---------------------------------------------------------------- L2 builder
@functools.lru_cache(maxsize=8)
def _build_l2(K, dt2, conv_f32r):
    """K: tuple of NSLOT ints -- tiles per PSUM row-slot (uniform across cores)."""
    T_u = sum(K)
    F32R = mybir.dt.float32r
    cdt = F32R if conv_f32r else F32
    nc = bacc.Bacc("TRN2", target_bir_lowering=False, debug=False, num_devices=NCORES)
    vals = nc.dram_tensor("vals", [128, T_u * CC], dt2, kind="ExternalInput")
    idx = nc.dram_tensor("idx", [128, T_u], F32, kind="ExternalInput")
    wt = nc.dram_tensor("wt", [CC, OC], cdt, kind="ExternalInput")
    sb = nc.dram_tensor("sb", [OC, 2], F32, kind="ExternalInput")
    iota_in = nc.dram_tensor("iota", [128, 128], dt2, kind="ExternalInput")
    y = nc.dram_tensor("y", [OC, NSLOT * BW], F32, kind="ExternalOutput")

    NCH = 4
    bnd = [0] + [((T_u * (i + 1)) // NCH) for i in range(NCH)]   # tile chunks

    with tile.TileContext(nc) as tc:
        with (
            tc.tile_pool(name="consts", bufs=1) as const_pool,
            tc.tile_pool(name="vals", bufs=2) as vals_pool,
            tc.tile_pool(name="oh", bufs=6) as oh_pool,
            tc.tile_pool(name="bevt", bufs=1) as bevt_pool,
            tc.tile_pool(name="yst", bufs=1) as yst_pool,
            tc.tile_pool(name="ps", bufs=4, space="PSUM") as ps_pool,
            tc.tile_pool(name="psy", bufs=2, space="PSUM") as psy_pool,
        ):
            iota_t = const_pool.tile([128, 128], dt2)
            nc.sync.dma_start(out=iota_t[:], in_=iota_in[:])
            idx_t = const_pool.tile([128, T_u], F32)
            nc.sync.dma_start(out=idx_t[:], in_=idx[:])
            wt_t = const_pool.tile([CC, OC], cdt)
            nc.sync.dma_start(out=wt_t[:], in_=wt[:])
            sb_t = const_pool.tile([OC, 2], F32)
            nc.sync.dma_start(out=sb_t[:], in_=sb[:])

            vt = []
            for chk in range(NCH):
                t = vals_pool.tile([128, (bnd[chk + 1] - bnd[chk]) * CC], dt2)
                nc.sync.dma_start(
                    out=t[:], in_=vals[:, bnd[chk] * CC:bnd[chk + 1] * CC]
                )
                vt.append(t)

            def val_slice(tf):
                chk = next(i for i in range(NCH) if bnd[i] <= tf < bnd[i + 1])
                lo = (tf - bnd[chk]) * CC
                return vt[chk][:, lo:lo + CC]

            bevt = bevt_pool.tile([CC, NSLOT * BW], cdt)
            tf = 0
            for s in range(NSLOT):
                ps = ps_pool.tile([CC, BW], F32, space="PSUM")
                for k in range(K[s]):
                    oh = oh_pool.tile([128, 128], dt2)
                    nc.vector.tensor_scalar(
                        out=oh[:],
                        in0=iota_t[:],
                        scalar1=idx_t[:, tf:tf + 1],
                        scalar2=None,
                        op0=mybir.AluOpType.is_equal,
                    )
                    nc.tensor.matmul(
                        out=ps[:],
                        lhsT=val_slice(tf),
                        rhs=oh[:],
                        start=(k == 0),
                        stop=(k == K[s] - 1),
                    )
                    tf += 1
                nc.scalar.copy(out=bevt[:, s * BW:(s + 1) * BW], in_=ps[:])

            yst = yst_pool.tile([OC, NSLOT * BW], F32)
            CONV_N = 512
            for q in range((NSLOT * BW) // CONV_N):
                psy = psy_pool.tile([OC, CONV_N], F32, space="PSUM")
                nc.tensor.matmul(
                    out=psy[:],
                    lhsT=wt_t[:],
                    rhs=bevt[:, q * CONV_N:(q + 1) * CONV_N],
                    start=True,
                    stop=True,
                )
                nc.scalar.activation(
                    out=yst[:, q * CONV_N:(q + 1) * CONV_N],
                    in_=psy[:],
                    func=mybir.ActivationFunctionType.Relu,
                    bias=sb_t[:, 1:2],
                    scale=sb_t[:, 0:1],
                )
            nc.sync.dma_start(out=y[:], in_=yst[:])
    nc.compile()
    return nc


# ---------------------------------------------------------------- host plan
def _compute_bins(intrinsics, cam2ego):
    """Mirror the reference's index math exactly (same jnp ops, same backend)
    so floor() results match bit-for-bit, then reduce over the h axis."""
    import jax.numpy as jnp

    intrinsics = jnp.asarray(intrinsics)
    cam2ego = jnp.asarray(cam2ego)
    u = ((jnp.arange(WF, dtype=jnp.float32) + 0.5) * STRIDE)[None, None, None, None, :]
    v = ((jnp.arange(HF, dtype=jnp.float32) + 0.5) * STRIDE)[None, None, None, :, None]
    Z = jnp.linspace(Z_MIN, Z_MAX, DD, dtype=jnp.float32)[None, None, :, None, None]

    fx = intrinsics[:, :, 0, 0][:, :, None, None, None]
    fy = intrinsics[:, :, 1, 1][:, :, None, None, None]
    cx = intrinsics[:, :, 0, 2][:, :, None, None, None]
    cy = intrinsics[:, :, 1, 2][:, :, None, None, None]

    Xc = (u - cx) / fx * Z
    Yc = (v - cy) / fy * Z
    Zc = jnp.broadcast_to(Z, Xc.shape)

    T = cam2ego[:, :, None, None, None]
    x_e = T[..., 0, 0] * Xc + T[..., 0, 1] * Yc + T[..., 0, 2] * Zc + T[..., 0, 3]
    y_e = T[..., 1, 0] * Xc + T[..., 1, 1] * Yc + T[..., 1, 2] * Zc + T[..., 1, 3]

    mx = (PC[3] - PC[0]) / BW
    my = (PC[4] - PC[1]) / BH
    ix = jnp.floor((x_e - PC[0]) / mx).astype(jnp.int32)
    iy = jnp.floor((y_e - PC[1]) / my).astype(jnp.int32)
    valid = (ix >= 0) & (ix < BW) & (iy >= 0) & (iy < BH)

    ix = np.asarray(ix)[0]
    iy = np.asarray(iy)[0]
    valid = np.asarray(valid)[0]
    # h-independence (holds for z-yaw-only rigs; required by this kernel)
    assert (ix == ix[:, :, :1, :]).all() and (iy == iy[:, :, :1, :]).all() and (
        valid == valid[:, :, :1, :]
    ).all(), "BEV bin depends on image row; kernel assumes z-yaw-only rig"
    return ix[:, :, 0, :], iy[:, :, 0, :], valid[:, :, 0, :]   # (N, D, W)


def _plan(intrinsics, cam2ego):
    ix, iy, valid = _compute_bins(intrinsics, cam2ego)
    # global point id = ray*DD + d, ray = n*WF + w
    ixr = ix.transpose(0, 2, 1).reshape(-1)      # (n, w, d) flattened
    iyr = iy.transpose(0, 2, 1).reshape(-1)
    vr = valid.transpose(0, 2, 1).reshape(-1)
    pid = np.arange(RAYS * DD, dtype=np.int64)

    vpid = pid[vr]
    vrow = iyr[vr].astype(np.int64)
    vcol = ixr[vr].astype(np.int64)

    # group points by BEV row
    order = np.argsort(vrow, kind="stable")
    vpid, vrow, vcol = vpid[order], vrow[order], vcol[order]
    rowcnt = np.bincount(vrow, minlength=BH)
    rowstart = np.concatenate([[0], np.cumsum(rowcnt)])
    tiles_per_row = np.maximum((rowcnt + 127) // 128, rowcnt > 0).astype(int)

    # snake-deal rows to cores by descending tile count -> 16 rows per core
    rorder = np.argsort(-tiles_per_row, kind="stable")
    core_rows = [[] for _ in range(NCORES)]
    for i, r in enumerate(rorder):
        rnd, pos = divmod(i, NCORES)
        c = pos if rnd % 2 == 0 else NCORES - 1 - pos
        core_rows[c].append(int(r))
    # per-core: rows sorted by tile count desc -> slot s
    for c in range(NCORES):
        core_rows[c].sort(key=lambda r: -tiles_per_row[r])
    K = tuple(
        int(max(tiles_per_row[core_rows[c][s]] for c in range(NCORES)))
        for s in range(NSLOT)
    )
    return dict(
        K=K,
        core_rows=core_rows,
        rowstart=rowstart,
        rowcnt=rowcnt,
        vpid=vpid,
        vcol=vcol,
    )


# ---------------------------------------------------------------- main entry
def _l1_inputs(depth_prob, context):
    dt = _np_dt(_DT1)
    # [h, ray, d] / [h, ray, c], depth padded d 48->64 with zeros
    dT = np.zeros((HF, RAYS, DP), np.float32)
    dT[:, :, :DD] = depth_prob[0].transpose(2, 0, 3, 1).reshape(HF, RAYS, DD)
    cT = np.ascontiguousarray(
        context[0].transpose(2, 0, 3, 1).reshape(HF, RAYS, CC)
    )
    maps = []
    for c in range(NCORES):
        sl = slice(c * RPC, (c + 1) * RPC)
        d = (
            dT[:, sl]
            .reshape(HF, RPC // 2, 2, DP)
            .transpose(2, 0, 1, 3)
            .reshape(128, -1)
            .astype(dt)
        )
        ct = (
            cT[:, sl]
            .reshape(HF, RPC // 2, 2, CC)
            .transpose(2, 0, 1, 3)
            .reshape(128, -1)
            .astype(dt)
        )
        maps.append({"d_in": d, "c_in": ct})
    return maps


def _l2_inputs(plan, part_all, w_proj, b_proj, bn_gamma, bn_beta, bn_mean, bn_var):
    dt = _np_dt(_DT2)
    K = plan["K"]
    T_u = sum(K)
    scale = (bn_gamma / np.sqrt(bn_var + BN_EPS)).astype(np.float32)
    bias = ((b_proj - bn_mean) * scale + bn_beta).astype(np.float32)
    sb = np.stack([scale, bias], axis=1)                     # (OC, 2)
    wt = np.ascontiguousarray(w_proj.T).astype(np.float32)   # (CC, OC)
    iota = np.arange(128, dtype=np.float32)[None, :].repeat(128, 0).astype(dt)

    rowstart, vpid, vcol = plan["rowstart"], plan["vpid"], plan["vcol"]
    maps = []
    for c in range(NCORES):
        vals = np.zeros((128, T_u, CC), np.float32)
        idx = np.full((128, T_u), -1.0, np.float32)
        tf = 0
        for s in range(NSLOT):
            r = plan["core_rows"][c][s]
            lo, hi = rowstart[r], rowstart[r + 1]
            pids = vpid[lo:hi]
            cols = vcol[lo:hi]
            for k in range(K[s]):
                seg = slice(k * 128, min((k + 1) * 128, hi - lo))
                n = max(0, seg.stop - seg.start)
                if n > 0:
                    vals[:n, tf] = part_all[pids[seg]]
                    idx[:n, tf] = cols[seg]
                tf += 1
        maps.append(
            {
                "vals": vals.reshape(128, -1).astype(dt),
                "idx": idx,
                "wt": wt,
                "sb": sb,
                "iota": iota,
            }
        )
    return maps


def kernel(**inputs) -> np.ndarray:
    depth_prob = np.asarray(inputs["depth_prob"], np.float32)
    context = np.asarray(inputs["context"], np.float32)
    intrinsics = np.asarray(inputs["intrinsics"], np.float32)
    cam2ego = np.asarray(inputs["cam2ego"], np.float32)

    plan = _plan(intrinsics, cam2ego)
    nc1 = _build_l1(_DT1)
    l1_maps = _l1_inputs(depth_prob, context)
    res1 = run_bass_kernel_spmd(nc1, l1_maps, list(range(NCORES))).results

    part_all = np.concatenate(
        [_unpack_l1(res1[c]["part"]) for c in range(NCORES)], axis=0
    ).reshape(RAYS * DD, CC)

    nc2 = _build_l2(plan["K"], _DT2, CONV_F32R)
    l2_maps = _l2_inputs(
        plan,
        part_all,
        np.asarray(inputs["w_proj"], np.float32),
        np.asarray(inputs["b_proj"], np.float32),
        np.asarray(inputs["bn_gamma"], np.float32),
        np.asarray(inputs["bn_beta"], np.float32),
        np.asarray(inputs["bn_mean"], np.float32),
        np.asarray(inputs["bn_var"], np.float32),
    )
    res2 = run_bass_kernel_spmd(nc2, l2_maps, list(range(NCORES))).results

    y = np.empty((1, OC, BH, BW), np.float32)
    for c in range(NCORES):
        yc = res2[c]["y"]                      # (OC, NSLOT*BW)
        for s in range(NSLOT):
            r = plan["core_rows"][c][s]
            y[0, :, r, :] = yc[:, s * BW:(s + 1) * BW]
    return y
